# revision 1
# baseline (speedup 1.0000x reference)
"""DisConv GNN message-passing kernel for 8 Trainium2 NeuronCores, rev 2.

Problem: Z = l2norm(features @ W_k + b_k); 4 iterations of
  att[k] = softmax_k(mask * (Z_k Z_k^T)); Z = l2norm(Z + att @ Z)
Output: [N, K*D] channel-concat.

Row sharding: N=2048 over 8 cores, 256 local columns each. Each core holds
replicated Z in bf16 in two layouts: ZTs (channel-major [32c x 2048m] stacks
of 4 channels) for score matmuls, Znm (n-major [128m x 16blk*8slot*32c]) for
aggregation matmuls.

Rev-2 changes vs rev 1 (which used 2 collectives + 16 DMAs per boundary,
~22us engine-idle per iteration):
- ONE AllGather per boundary: agin [4,128,256] packs ztl (stacks 0,1) and
  znml (col-halves 0,1); one agin DMA from a packed SBUF tile; 6 reload
  DMAs (4 ZT rank-quad + 2 Znm rank-quad, interleaved). HWDGE fixed cost
  is ~625ns/DMA on a shared device and a collective costs ~10-15us wall,
  so boundary serialization drops by roughly half.
- score_tiles=2: two [128,1024] f32 PSUM score tiles per block -> 2 exps of
  1024 cols instead of 4 of 512 (ACT fixed access latency ~185ns/op).
- Norm-squared matmul in bf16 (f32 PE matmuls run at 4 cyc/col; bf16 at 1).
- Init projection in bf16; mask loaded in one strided DMA on the scalar
  queue early.
- Softmax restructuring as rev 1: softmax input masking is k-independent,
  so att = mask * exp(S) / sum_k exp(S) matches the reference exactly.
- l2norm via rnorm = exp(-0.5*ln(s^2+eps)); Exp/Ln pinned to the
  natural_log_exp_and_others ACT table set (one table load total).
- HW-verified constraint kept: concurrent PE row groups write distinct PSUM
  banks (score tile h holds channels 2h,2h+1; within a [128,1024] f32 tile
  row group parity picks the bank).
- Software-pipelined emission (den_lag/agg_lag): engines execute their
  queues in order, so block b's rcp (waiting on the Pool t2->den round
  trip) at the DVE queue head would stall block b+1's t1. Emitting
  rcp/rmask/att one block late and aggs three blocks late keeps every
  queue head dep-satisfied; cost-model block cadence drops 2.86 -> 2.6us.
- DMA AP structure rule (hardware-debugged): a DMA whose DRAM-side dims
  merge (e.g. (t p) contiguous) while the SBUF side cannot mispairs
  descriptor dims and writes garbage -- keep both sides' AP dim structures
  identical (iterate partition-major).

Measured (rep-slope K1-vs-K49, see test.py): 303/361/457us med-slopes
across three windows (session relay noise dominates the spread), with the
rev-1 baseline at 525us in the SAME window as the 457 (13% faster under
identical conditions) and 410us recorded in its own quiet session.
TimelineSim (loopback stand-in) predicts 277us vs 338us for rev 1.
"""

import sys

sys.path.insert(0, "/opt/trn_rl_repo")

import numpy as np
import ml_dtypes

N = 2048
IN_DIM = 128
K = 8
D = 32
ITERS = 4
NCORES = 8
NLOC = N // NCORES  # 256
NBLK = N // 128  # 16
EPS2 = 1e-24

BF = ml_dtypes.bfloat16

_compiled = None


DEFAULT_CFG = dict(
    lvl2_eng="gpsimd",
    den_eng="gpsimd",
    rm_eng="vector",
    eall_bufs=5,
    att_bufs=3,
    pipe_bufs=5,
    att_split=2,
    score_imajor=True,
    reload_big=False,  # True: 2 ZT + 2 Znm reload DMAs; False: 4 + 4
    znm_full=True,  # True: 2 full-range Znm reload DMAs instead of 4
    copy_eng="mixed",  # znml PSUM->SBUF copies: "dve" | "mixed" (2 DVE + 2 ACT)
    tail_fast=0,  # last tail_fast blocks run lvl2/den on DVE (shorter drain)
    # Emit block b's aggregation matmuls after block b+agg_lag's scores. PE
    # executes in order; with lag 0 the exp->tree->rcp->att->agg chain (~5us
    # with sem hops) must fit in ~2 block periods. Lag L stretches that to
    # L+2 periods so the ACT/DVE throughput bound sets the block cadence.
    agg_lag=3,
    # Emit block b's rcp/rmask/att after block b+den_lag's den, so the rcp
    # at the DVE queue head never waits on the Pool t2->den round trip.
    den_lag=1,
    # Emit the lagged stage1 at the TOP of each block (before this block's
    # scores/exps/t1) instead of after its den. Measured worse: with
    # den_lag=1 the rcp then reaches the DVE head before its den lands.
    stage1_first=False,
    # sim-only ablations (break numerics; for bottleneck bisection)
    skip_tree=False,
    skip_att=False,
    skip_agg=False,
    skip_exp=False,
)


def _patch_act_tables(bacc, mybir):
    # The ACT table-load pass picks the first set containing each function,
    # which puts Exp (set 0) and Ln (set 5) in different table sets and
    # reloads tables twice per iteration boundary. Restrict Exp/Ln to
    # natural_log_exp_and_others (which holds both) so one load serves the
    # whole kernel. Indices/order are preserved.
    if getattr(bacc, "_dis_act_tables_patched", False):
        return
    _orig_tabs = bacc.get_activation_tables

    def _patched_tabs(arch, _orig=_orig_tabs, _AF=mybir.ActivationFunctionType):
        out = {}
        for name, fns in _orig(arch).items():
            fns = set(fns)
            if name != "natural_log_exp_and_others":
                fns.discard(_AF.Exp)
                fns.discard(_AF.Ln)
            out[name] = fns
        return out

    bacc.get_activation_tables = _patched_tabs
    bacc._dis_act_tables_patched = True


def _build(reps=1, sim_mode=False, cfg=None):
    """sim_mode: single-core, collective replaced by 8 DRAM->DRAM loopback
    DMAs with the same dependency shape, for TimelineSim iteration."""
    import concourse.bacc as bacc
    import concourse.mybir as mybir
    from concourse import tile

    _patch_act_tables(bacc, mybir)
    cfg = {**DEFAULT_CFG, **(cfg or {})}
    # att tile for block b is read by aggs emitted at block b+agg_lag;
    # eall for block b is read by att at block b+den_lag; den likewise.
    cfg["agg_lag"] = max(cfg["agg_lag"], cfg["den_lag"])
    cfg["att_bufs"] = max(cfg["att_bufs"], cfg["agg_lag"] - cfg["den_lag"] + 3)
    cfg["eall_bufs"] = max(cfg["eall_bufs"], cfg["den_lag"] + 3)
    cfg["pipe_bufs"] = max(cfg["pipe_bufs"], cfg["den_lag"] + 2)

    f32 = mybir.dt.float32
    bf16 = mybir.dt.bfloat16
    AF = mybir.ActivationFunctionType
    ALU = mybir.AluOpType

    nc = bacc.Bacc(
        "TRN2",
        target_bir_lowering=False,
        debug=False,
        num_devices=1 if sim_mode else NCORES,
    )
    nc._dis_sim_mode = sim_mode
    nc._dis_cfg = cfg

    # ---- I/O -------------------------------------------------------------
    # featw packs featT_loc (cols 0:256) and wstack (cols 256:512); consts
    # packs bstack (cols 0:2) and id128 (cols 2:130).
    featw_in = nc.dram_tensor("featw", [IN_DIM, NLOC + K * D], bf16, kind="ExternalInput")
    maskT_in = nc.dram_tensor("maskT", [NBLK, 128, NLOC], bf16, kind="ExternalInput")
    consts_in = nc.dram_tensor("consts", [128, 130], f32, kind="ExternalInput")
    onesblk_in = nc.dram_tensor("onesblk", [128, 128], bf16, kind="ExternalInput")
    out_dram = nc.dram_tensor("out", [2, 128, NLOC], f32, kind="ExternalOutput")
    dbg_dram = None
    if cfg.get("debug_dump"):
        dbg_dram = nc.dram_tensor("dbg", [4, 128, N], bf16, kind="ExternalOutput")
    nc._dis_dbg = dbg_dram

    rg = [list(range(NCORES))]

    with tile.TileContext(nc) as tc:
        with (
            tc.tile_pool(name="const", bufs=1) as constp,
            tc.tile_pool(name="state", bufs=2) as statep,
            tc.tile_pool(name="work", bufs=2) as workp,
            tc.tile_pool(name="psum", bufs=1, space="PSUM") as psp,
            tc.tile_pool(name="psagg", bufs=1, space="PSUM") as psaggp,
            tc.tile_pool(name="dram", bufs=2, space="DRAM") as dramp,
        ):
            # ---- persistent SBUF tensors --------------------------------
            # mask first, on the scalar queue: one big strided DMA, not
            # needed until the first block's rmask.
            maskT = constp.tile([128, NBLK * NLOC], bf16)
            nc.scalar.dma_start(
                maskT[:].rearrange("p (b n) -> p b n", b=NBLK),
                maskT_in[:].rearrange("b p n -> p b n"),
            )
            featw = constp.tile([IN_DIM, NLOC + K * D], bf16)
            nc.sync.dma_start(featw[:], featw_in[:])
            consts = constp.tile([128, 130], f32)
            nc.sync.dma_start(consts[:], consts_in[:])
            onesblk = constp.tile([128, 128], bf16)
            nc.sync.dma_start(onesblk[:], onesblk_in[:])
            epsb = constp.tile([128, 1], f32)
            nc.any.memset(epsb[:], EPS2)

            # replicated Z (bf16, rebuilt each round via one AllGather)
            ZTs = [constp.tile([128, N], bf16, name=f"ZT{s}") for s in range(2)]
            Znm = constp.tile([128, NBLK * K * D], bf16)

            def norm_dist(zsum, rnd, last):
                """zsum: 2 stacks [128(4ch x 32c), NLOC] f32. Normalizes,
                produces new local f32 state + the packed bf16 agin tile
                (ztl at cols s*256, znml at cols 512+c*256), runs ONE
                AllGather, reloads ZT/Znm replicas. If last, writes the
                output instead and returns (zloc, None)."""
                # per-stack pipelined norm: stack 0's aggs stop ~0.4us before
                # stack 1's, and splitting sq/n2/Ln/Exp per stack lets stack
                # 0's zloc/transposes overlap stack 1's chain on other engines.
                sq = workp.tile([128, 2 * NLOC], bf16, name=f"sq{rnd}", tag="sq")
                n2 = psp.tile([128, 2 * NLOC], f32, name=f"n2{rnd}", tag="sps0")
                lg = workp.tile([128, 2 * NLOC], f32, name=f"lg{rnd}", tag="lg")
                rn = workp.tile([128, 2 * NLOC], f32, name=f"rn{rnd}", tag="rn", bufs=3)
                for s in range(2):
                    sl = slice(s * NLOC, (s + 1) * NLOC)
                    nc.vector.tensor_tensor(sq[:, sl], zsum[s][:], zsum[s][:], ALU.mult)
                for s in range(2):
                    sl = slice(s * NLOC, (s + 1) * NLOC)
                    nc.tensor.matmul(
                        n2[:, sl], onesblk[:], sq[:, sl], start=True, stop=True
                    )
                    nc.scalar.activation(lg[:, sl], n2[:, sl], AF.Ln, bias=epsb[:])
                    nc.scalar.activation(rn[:, sl], lg[:, sl], AF.Exp, scale=-0.5)
                zloc = []
                for s in range(2):
                    sl = slice(s * NLOC, (s + 1) * NLOC)
                    zn = statep.tile(
                        [128, NLOC], f32, name=f"zloc{rnd}{s}", tag=f"zloc{s}"
                    )
                    nc.vector.tensor_tensor(zn[:], zsum[s][:], rn[:, sl], ALU.mult)
                    zloc.append(zn)

                # transpose local columns to n-major PSUM. Concurrent PE row
                # groups must write distinct PSUM banks: row group i writes
                # tile i//2, bank parity i%2. Col layout inside tile:
                # (i%2)*512 + c*128 + s*32 (channel 4s+i -> slot 2i+s).
                pt = [
                    psp.tile([128, 4 * NLOC], f32, name=f"pt{rnd}{h}", tag=f"sps{h}")
                    for h in range(2)
                ]

                def tslice(i, s, c, w=32):
                    base = (i % 2) * 2 * NLOC + c * 128 + s * 32
                    return pt[i // 2][:, base : base + w]

                # s-major: all stack-0 transposes first (PE executes in
                # order; stack 1's zloc lands ~0.4us later)
                for s in range(2):
                    for c in range(2):
                        for i in range(4):
                            nc.tensor.transpose(
                                tslice(i, s, c),
                                zloc[s][32 * i : 32 * (i + 1), c * 128 : (c + 1) * 128],
                                consts[32 * i : 32 * (i + 1), 2 + 32 * i : 2 + 32 * (i + 1)],
                                tile_position=(32 * i, 0),
                            )

                if last:
                    # output col order is channel-major k=4s+i: cols
                    # s*128 + i*32; pt banks hold (i parity) at col s*32.
                    for c in range(2):
                        ot = workp.tile([128, 256], f32, name=f"ot{c}", tag="ot")
                        for h in range(2):
                            for s in range(2):
                                dst = ot[
                                    :, s * 128 + h * 64 : s * 128 + (h + 1) * 64
                                ].rearrange("p (b w) -> p b w", b=2)
                                src = pt[h][:].rearrange("p (b x) -> p b x", b=2)[
                                    :, :, c * 128 + s * 32 : c * 128 + (s + 1) * 32
                                ]
                                if h == 1:
                                    nc.scalar.copy(dst, src)
                                else:
                                    nc.vector.tensor_copy(dst, src)
                        nc.sync.dma_start(out_dram[c], ot[:])
                    return zloc, None

                # packed agin tile: cols [s*256] = ztl stack s (channel-major
                # local bf16 z), cols [512 + c*256 + i*64] = znml col-half c.
                agin_sb = workp.tile([128, 1024], bf16, name=f"agin{rnd}", tag="agin")
                for s in range(2):
                    nc.vector.tensor_tensor(
                        agin_sb[:, s * NLOC : (s + 1) * NLOC],
                        zsum[s][:],
                        rn[:, s * NLOC : (s + 1) * NLOC],
                        ALU.mult,
                    )
                # znml copies: per (tile h, col-half c) one strided copy
                # [128, 2 banks, 64] covering slots 4h..4h+3 of half c.
                for h in range(2):
                    for c in range(2):
                        dst = agin_sb[
                            :, 512 + c * 256 + h * 128 : 512 + c * 256 + (h + 1) * 128
                        ].rearrange("p (b w) -> p b w", b=2)
                        src = pt[h][:].rearrange("p (b x) -> p b x", b=2)[
                            :, :, c * 128 : c * 128 + 64
                        ]
                        if cfg["copy_eng"] == "mixed" and h == 1:
                            nc.scalar.copy(dst, src)
                        else:
                            nc.vector.tensor_copy(dst, src)

                sim = getattr(nc, "_dis_sim_mode", False)
                shared = "Local" if sim else "Shared"
                agin = dramp.tile([4, 128, 256], bf16, name=f"agin_{rnd}", tag="agind")
                # iterate both sides partition-major so the AP dim structures
                # match ([128,4,256] on each side); a (t p) merge on the DRAM
                # side against an unmergeable SBUF side mispairs descriptors.
                nc.sync.dma_start(
                    agin[:].rearrange("t p n -> p t n"),
                    agin_sb[:].rearrange("p (t n) -> p t n", t=4),
                )
                nc._dis_agin = agin
                agout = dramp.tile(
                    [NCORES, 4, 128, 256], bf16,
                    name=f"agout_{rnd}", tag="agout", addr_space=shared,
                )
                if sim:  # stand-in with the same dependency shape
                    for r in range(NCORES):
                        nc.scalar.dma_start(agout[r], agin[:])
                else:
                    nc.gpsimd.collective_compute(
                        "AllGather",
                        mybir.AluOpType.bypass,
                        replica_groups=rg,
                        ins=[agin[:].opt()],
                        outs=[agout[:].opt()],
                    )

                # reloads: ZT before Znm so early blocks' scores start first.
                if cfg["reload_big"]:
                    for s in range(2):
                        nc.sync.dma_start(
                            ZTs[s][:].rearrange("p (r n) -> p r n", r=8),
                            agout[:, s].rearrange("r p n -> p r n"),
                        )
                    znm_v = Znm[:].rearrange("p (r c n) -> p r c n", r=8, c=2)
                    for c in range(2):
                        nc.sync.dma_start(
                            znm_v[:, :, c, :],
                            agout[:, 2 + c].rearrange("r p n -> p r n"),
                        )
                elif cfg["znm_full"]:
                    for g in range(2):
                        for s in range(2):
                            nc.sync.dma_start(
                                ZTs[s][:, g * 1024 : (g + 1) * 1024].rearrange(
                                    "p (r n) -> p r n", r=4
                                ),
                                agout[4 * g : 4 * g + 4, s].rearrange("r p n -> p r n"),
                            )
                    znm_v = Znm[:].rearrange("p (r c n) -> p r c n", r=8, c=2)
                    for c in range(2):
                        nc.sync.dma_start(
                            znm_v[:, :, c, :],
                            agout[:, 2 + c].rearrange("r p n -> p r n"),
                        )
                else:
                    for g in range(2):
                        for s in range(2):
                            nc.sync.dma_start(
                                ZTs[s][:, g * 1024 : (g + 1) * 1024].rearrange(
                                    "p (r n) -> p r n", r=4
                                ),
                                agout[4 * g : 4 * g + 4, s].rearrange("r p n -> p r n"),
                            )
                        znm_g = Znm[:, g * 2048 : (g + 1) * 2048].rearrange(
                            "p (r c n) -> p r c n", r=4, c=2
                        )
                        for c in range(2):
                            nc.sync.dma_start(
                                znm_g[:, :, c, :],
                                agout[4 * g : 4 * g + 4, 2 + c].rearrange(
                                    "r p n -> p r n"
                                ),
                            )
                return zloc, agin_sb

            for rep in range(reps):
                _body_once(
                    nc, tc, tile, mybir, rep, cfg,
                    featw, consts, maskT, ZTs, Znm,
                    statep, workp, psp, psaggp, norm_dist,
                )

    nc.compile()
    return nc


def _body_once(
    nc, tc, tile, mybir, rep, cfg,
    featw, consts, maskT, ZTs, Znm,
    statep, workp, psp, psaggp, norm_dist,
):
    f32 = mybir.dt.float32
    bf16 = mybir.dt.bfloat16
    AF = mybir.ActivationFunctionType
    ALU = mybir.AluOpType

    # ---- init: Z0 = l2norm(features @ W + b) for local columns (bf16) ----
    zsum0 = []
    for s in range(2):
        ip = psp.tile([128, NLOC], f32, name=f"initp{rep}{s}", tag=f"sps{s}")
        for i in range(4):
            nc.tensor.matmul(
                ip[32 * i : 32 * (i + 1), :],
                featw[:, NLOC + (4 * s + i) * D : NLOC + (4 * s + i + 1) * D],
                featw[:, 0:NLOC],
                start=True,
                stop=True,
                tile_position=(0, 32 * i),
            )
        zs = workp.tile([128, NLOC], f32, name=f"zsum0{rep}{s}", tag="zsum")
        nc.vector.tensor_scalar(zs[:], ip[:], consts[:, s : s + 1], None, ALU.add)
        zsum0.append(zs)
    zloc, agin_sb = norm_dist(zsum0, f"{rep}_0", last=(ITERS == 0))

    # ---- iterations ------------------------------------------------------
    dbg = getattr(nc, "_dis_dbg", None)
    if dbg is not None:
        nc.sync.dma_start(dbg[0], ZTs[0][:])
        nc.sync.dma_start(dbg[1], ZTs[1][:])
        nc.sync.dma_start(
            dbg[2][:, 0:1024].rearrange("p (t n) -> p t n", t=4),
            nc._dis_agin[:].rearrange("t p n -> p t n"),
        )
        nc.sync.dma_start(dbg[3][:, 0:1024], agin_sb[:])
    lag = cfg["agg_lag"]
    for it in range(ITERS):
        aggps = [
            psaggp.tile([128, NLOC], f32, name=f"agg{rep}{it}{s}", tag=f"agg{s}")
            for s in range(2)
        ]
        att_q = {}

        def emit_aggs(b):
            att_b = att_q.pop(b)
            if cfg["skip_agg"]:
                return
            for s in range(2):
                for i in range(4):
                    slot = 2 * i + s  # channel 4s+i in replica layouts
                    nc.tensor.matmul(
                        aggps[s][32 * i : 32 * (i + 1), :],
                        Znm[:, (b * K + slot) * D : (b * K + slot + 1) * D],
                        att_b[:, slot * NLOC : (slot + 1) * NLOC],
                        start=(b == 0),
                        stop=(b == NBLK - 1),
                        tile_position=(0, 32 * i),
                        skip_group_check=True,
                    )

        den_q = {}

        def stage1(b):
            eall, den = den_q.pop(b)
            rcp = workp.tile(
                [128, NLOC], f32, name=f"rcp{rep}{it}{b}", tag="rcp",
                bufs=cfg["pipe_bufs"],
            )
            nc.vector.reciprocal_approx_fast(rcp[:], den[:])
            rmask = workp.tile(
                [128, NLOC], bf16, name=f"rm{rep}{it}{b}", tag="rm",
                bufs=cfg["pipe_bufs"],
            )
            engr = nc.gpsimd if cfg["rm_eng"] == "gpsimd" else nc.vector
            engr.tensor_tensor(
                rmask[:], rcp[:], maskT[:, b * NLOC : (b + 1) * NLOC], ALU.mult
            )
            att = workp.tile(
                [128, K * NLOC], bf16, name=f"att{rep}{it}{b}", tag="att",
                bufs=cfg["att_bufs"],
            )
            nsp = cfg["att_split"]
            kk = K // nsp
            for h in range(nsp):
                lo = h * kk * NLOC
                hi = (h + 1) * kk * NLOC
                nc.vector.tensor_tensor(
                    att[:, lo:hi].rearrange("p (a n) -> p a n", a=kk),
                    eall[:, lo:hi].rearrange("p (a n) -> p a n", a=kk),
                    rmask[:, None, :].to_broadcast((128, kk, NLOC)),
                    ALU.mult,
                )
            att_q[b] = att

        dlag = cfg["den_lag"]
        for blk in range(NBLK):
            # Lagged stage1 FIRST: rcp/rmask/att of block blk-dlag are
            # dep-ready now; emitting them before this block's t1 keeps the
            # DVE queue head satisfied while ACT computes this block's exps
            # (t1 at the head would stall the whole DVE queue on exp_blk).
            if cfg["stage1_first"] and not cfg["skip_tree"] and blk >= dlag:
                stage1(blk - dlag)
            # Score tiles: tile h holds channels i=2h,2h+1 (slots 4h..4h+3,
            # channel (s,i) at slot 2i+s). Row group i writes bank i%2 of
            # tile i//2.
            sps = [
                psp.tile(
                    [128, 4 * NLOC], f32, name=f"sps{rep}{it}{blk}{h}", tag=f"sps{h}"
                )
                for h in range(2)
            ]

            def sslice(s, i):
                base = (i % 2) * 2 * NLOC + s * NLOC
                return sps[i // 2][:, base : base + NLOC]

            order = (
                [(s, i) for i in range(4) for s in range(2)]
                if cfg["score_imajor"]
                else [(s, i) for s in range(2) for i in range(4)]
            )
            for s, i in order:
                nc.tensor.matmul(
                    sslice(s, i),
                    ZTs[s][32 * i : 32 * (i + 1), blk * 128 : (blk + 1) * 128],
                    agin_sb[32 * i : 32 * (i + 1), s * NLOC : (s + 1) * NLOC],
                    start=True,
                    stop=True,
                    tile_position=(32 * i, 0),
                )
            eall = workp.tile(
                [128, K * NLOC], bf16, name=f"eall{rep}{it}{blk}", tag="eall",
                bufs=cfg["eall_bufs"],
            )
            for h in range(2):
                nc.scalar.activation(
                    eall[:, h * 1024 : (h + 1) * 1024], sps[h][:], AF.Exp
                )
            if cfg["skip_tree"]:
                att_q[blk] = eall
            else:
                # denominator tree: 3 ops
                t1 = workp.tile(
                    [128, 1024], bf16, name=f"t1_{rep}{it}{blk}", tag="t1",
                    bufs=cfg["pipe_bufs"],
                )
                ev = eall[:].rearrange("p (a n) -> p a n", a=4)
                t1v = t1[:].rearrange("p (a n) -> p a n", a=4)
                nc.vector.tensor_tensor(
                    t1v, ev[:, :, 0:NLOC], ev[:, :, NLOC : 2 * NLOC], ALU.add
                )
                tail = blk >= NBLK - cfg["tail_fast"]
                t2 = workp.tile(
                    [128, 512], bf16, name=f"t2_{rep}{it}{blk}", tag="t2",
                    bufs=cfg["pipe_bufs"],
                )
                t1w = t1[:].rearrange("p (a n) -> p a n", a=2)
                t2v = t2[:].rearrange("p (a n) -> p a n", a=2)
                eng2 = (
                    nc.gpsimd if cfg["lvl2_eng"] == "gpsimd" and not tail else nc.vector
                )
                eng2.tensor_tensor(
                    t2v, t1w[:, :, 0:NLOC], t1w[:, :, NLOC : 2 * NLOC], ALU.add
                )
                den = workp.tile(
                    [128, NLOC], f32, name=f"den{rep}{it}{blk}", tag="den",
                    bufs=cfg["pipe_bufs"],
                )
                engd = (
                    nc.gpsimd if cfg["den_eng"] == "gpsimd" and not tail else nc.vector
                )
                engd.tensor_tensor(
                    den[:], t2[:, 0:NLOC], t2[:, NLOC : 2 * NLOC], ALU.add
                )
                den_q[blk] = (eall, den)
                if not cfg["stage1_first"] and blk >= dlag:
                    stage1(blk - dlag)
            if blk >= lag:
                emit_aggs(blk - lag)
        for b in range(NBLK - dlag, NBLK):
            if not cfg["skip_tree"]:
                stage1(b)
        for b in range(NBLK - lag, NBLK):
            emit_aggs(b)
        # residual + renorm + redistribute
        zsum = []
        for s in range(2):
            zs = workp.tile([128, NLOC], f32, name=f"zsum{rep}{it}{s}", tag="zsum")
            agg_src = zloc[s][:] if cfg["skip_agg"] else aggps[s][:]
            nc.vector.tensor_tensor(zs[:], zloc[s][:], agg_src, ALU.add)
            zsum.append(zs)
        zloc, agin_sb = norm_dist(zsum, f"{rep}_{it + 1}", last=(it == ITERS - 1))


def _prep_inputs(adj, features, W, b):
    adj = np.asarray(adj)
    features = np.asarray(features, np.float32)
    W = np.asarray(W, np.float32)
    b = np.asarray(b, np.float32)

    wstack = np.ascontiguousarray(
        W.transpose(1, 0, 2).reshape(IN_DIM, K * D)
    ).astype(BF)
    consts = np.zeros((128, 130), np.float32)
    for s in range(2):
        for i in range(4):
            consts[32 * i : 32 * (i + 1), s] = b[4 * s + i]
    consts[:, 2:130] = np.eye(128, dtype=np.float32)
    onesblk = np.zeros((128, 128), np.float32)
    for j in range(4):
        onesblk[32 * j : 32 * (j + 1), 32 * j : 32 * (j + 1)] = 1.0
    onesblk = onesblk.astype(BF)

    in_maps = []
    for c in range(NCORES):
        rows = slice(c * NLOC, (c + 1) * NLOC)
        featw = np.concatenate(
            [np.ascontiguousarray(features[rows].T).astype(BF), wstack], axis=1
        )
        maskT = (adj[rows].T > 0).astype(np.float32).astype(BF)
        maskT = np.ascontiguousarray(maskT.reshape(NBLK, 128, NLOC))
        in_maps.append(
            {
                "featw": featw,
                "maskT": maskT,
                "consts": consts,
                "onesblk": onesblk,
            }
        )
    return in_maps


def run(adj, features, W, b, trace=False, **trace_kwargs):
    global _compiled
    if _compiled is None:
        _compiled = _build()
    from concourse import bass_utils

    in_maps = _prep_inputs(adj, features, W, b)
    res = bass_utils.run_bass_kernel_spmd(
        _compiled, in_maps, core_ids=list(range(NCORES)), trace=trace, **trace_kwargs
    )
    outs = [res.results[c]["out"].reshape(NLOC, NLOC) for c in range(NCORES)]
    full = np.concatenate(outs, axis=0)
    return full, res


def kernel(adj, features, W, b):
    full, _ = run(adj, features, W, b, trace=False)
    return full

